# Initial kernel scaffold
#
"""Multi-head self-attention (B=2, S=2048, E=1024, H=16, D=64) on 8 TRN2 cores.

Sharding: core c handles batch b = c // 4 and heads (c % 4)*4 .. +4 (data
parallel over batch x tensor parallel over heads). Each core computes a
partial output projection over its 4 heads' slice of the residual; the host
sums the 4 fp16 partials per batch in fp32.

v3 kernel structure (chunk-major flash pipeline):
  - fp16 matmul datapath, fp32 PSUM accumulation, exp with -4 bias.
  - QKV projection is chunk-major: attention for query-chunk c starts as soon
    as projections for chunks <= c are done; projections for chunk c+1 and
    output-projection tiles are interleaved into the attention stream as PE
    filler, which also keeps the HAM clock gate open through the ACT-bound
    stretches.
  - V is projected token-major directly (x tile stationary, w_v moving): no
    PE transposes. V tiles for chunk c are produced early in attention(c).
  - Score matmuls have K=64 (head dim): the two heads of a pair live at
    partitions 0-63 / 64-127, so their score matmuls auto-derive PE row-tile
    positions (0,0)/(64,0) and execute concurrently on the array. Both heads
    score into ONE two-bank psum tile so a single ACTIVATE does both exps
    (the ~352-cycle ACTIVATE overhead is the scalar-engine tax).
  - Causality at 128-column granularity everywhere: scores, exp, and the PV
    accumulation are restricted to [off:] on diagonal key tiles.
  - Softmax denominator comes out of the PV matmul for free (ones column in
    the stationary). reciprocal_approx_fast + DRAM-bounce partition
    broadcast, deferred so the vector queue never waits on the round trip.
    Broadcast/output DMAs are triggered from the idle GPSIMD queue.
  - All weight/activation DMAs are host-packed so each is one plain 2D
    descriptor (descriptor generation on the Sync queue was a startup
    bottleneck).
"""

import sys
import types

import numpy as np


def _ensure_ntff_hook():
    """Install the axon NTFF profiling hook if the image's antenv lacks it."""
    try:
        import antenv.axon_hooks  # noqa: F401

        return
    except ImportError:
        pass
    try:
        import antenv
    except ImportError:
        return
    mod = types.ModuleType("antenv.axon_hooks")
    state = {"hook": None}
    mod.set_axon_ntff_profile_hook = lambda h: state.__setitem__("hook", h)
    mod.get_axon_ntff_profile_hook = lambda: state["hook"]
    sys.modules["antenv.axon_hooks"] = mod
    antenv.axon_hooks = mod
    try:
        from trn_agent_boot.trn_boot import _ntff_profile_via_ctypes

        mod.set_axon_ntff_profile_hook(
            _ntff_profile_via_ctypes("/opt/axon/libaxon_pjrt.so")
        )
    except Exception:
        pass


_ensure_ntff_hook()

import concourse.bass as bass
import concourse.mybir as mybir
import concourse.tile as tile
from concourse import bacc
from concourse import bass_utils as _bass_utils
from concourse.bass_utils import run_bass_kernel_spmd

_orig_upload = _bass_utils.upload_artifacts


def _safe_upload(tmpdir):
    try:
        return _orig_upload(tmpdir)
    except Exception:
        return ""


_bass_utils.upload_artifacts = _safe_upload

F32 = mybir.dt.float32
F16 = mybir.dt.float16

B, S, E, H, D = 2, 2048, 1024, 16, 64
NCORES = 8
CPB = 4            # cores per batch
HPC = H // CPB     # heads per core = 4
T = S              # tokens per core (one batch)
P = 128
QCH = 512          # query-chunk (psum free dim)
NCH = T // QCH     # 4 query chunks
NKT = T // P       # 16 key tiles
KO = E // P        # 8 contraction subtiles for the projections
NFT = 4            # q01, q23, k01, k23 feature tiles (v is token-major)
SCALE = 1.0 / np.sqrt(D)
EXP_BIAS = -4.0    # keeps exp() in fp16 range; cancels in softmax


def build_nc() -> bass.Bass:
    nc = bacc.Bacc(None, target_bir_lowering=False)
    xR_d = nc.dram_tensor("xR", [P, NCH, KO, QCH], F16, kind="ExternalInput")
    wT_d = nc.dram_tensor("wT", [P, KO, NFT * P], F16, kind="ExternalInput")
    wvT_d = nc.dram_tensor("wvT", [P, KO, HPC * D], F16, kind="ExternalInput")
    w0T_d = nc.dram_tensor("w0T", [P, HPC // 2, E], F16, kind="ExternalInput")
    mask_d = nc.dram_tensor("mask", [P, 4, QCH], F16, kind="ExternalInput")
    vones_d = nc.dram_tensor("vones", [P, NKT], F16, kind="ExternalInput")
    out_d = nc.dram_tensor("out", [T, E], F16, kind="ExternalOutput")

    with tile.TileContext(nc) as tc, nc.allow_low_precision(
        reason="fp16 matmul datapath by design; fp32 PSUM accumulation"
    ):
        _body(tc, xR_d, wT_d, wvT_d, w0T_d, mask_d, vones_d, out_d)
    nc.finalize()
    return nc


def _body(tc, xR_d, wT_d, wvT_d, w0T_d, mask_d, vones_d, out_d):
    nc = tc.nc
    with (
        tc.tile_pool(name="singles", bufs=1) as singles,
        tc.tile_pool(name="xchunks", bufs=2) as xchunks,
        tc.tile_pool(name="ptiles", bufs=4) as ptiles,
        tc.tile_pool(name="rtiles", bufs=4) as rtiles,
        tc.tile_pool(name="otiles", bufs=4) as otiles,
        tc.tile_pool(name="dpool", bufs=8, space="DRAM") as dpool,
        tc.tile_pool(name="stage", bufs=4) as stage,
        tc.tile_pool(name="ps_mm", bufs=2, space="PSUM") as ps_mm,
        tc.tile_pool(name="ps_s", bufs=2, space="PSUM") as ps_s,
        tc.tile_pool(name="ps_pv", bufs=2, space="PSUM") as ps_pv,
    ):
        # warmup matmuls on a zeroed tile open the HAM clock gate while the
        # DMA preamble streams in (DVE memset is the only dependency)
        wz = singles.tile([P, P], F16)
        nc.vector.memset(wz[:], 0.0)
        for w in range(8):
            wps = ps_mm.tile([P, P], F32, tag="mm512", name=f"warm{w}")
            nc.tensor.matmul(wps[:], wz[:], wz[:], start=True, stop=True)

        # preamble DMAs are split into ko-pair pieces spread over the three
        # DMA-capable engine queues so the transfers run in parallel and the
        # first projection matmuls can start as soon as their slices land
        w_sb = singles.tile([P, KO, NFT * P], F16)
        wv_sb = singles.tile([P, KO, HPC * D], F16)
        xcs = [None] * NCH
        xc0 = xchunks.tile([P, KO, QCH], F16, tag="xc", name="xc0")
        xcs[0] = xc0
        engs = [nc.sync, nc.scalar, nc.gpsimd]
        for i in range(KO // 2):
            s = slice(2 * i, 2 * i + 2)
            engs[(2 * i) % 3].dma_start(w_sb[:, s, :], wT_d[:, s, :])
            engs[(2 * i + 1) % 3].dma_start(xc0[:, s, :], xR_d[:, 0, s, :])
        mask_sb = singles.tile([P, 4, QCH], F16)
        nc.gpsimd.dma_start(mask_sb[:], mask_d[:])
        nc.scalar.dma_start(wv_sb[:], wvT_d[:])
        w0_sb = singles.tile([P, HPC // 2, E], F16)
        nc.gpsimd.dma_start(w0_sb[:], w0T_d[:])

        def dma_x_chunk(c):
            xc = xchunks.tile([P, KO, QCH], F16, tag="xc", name=f"xc{c}")
            nc.sync.dma_start(xc[:, : KO // 2, :], xR_d[:, c, : KO // 2, :])
            nc.sync.dma_start(xc[:, KO // 2 :, :], xR_d[:, c, KO // 2 :, :])
            xcs[c] = xc

        ebias = singles.tile([P, 1], F32)
        nc.vector.memset(ebias[:], EXP_BIAS)
        # trigger the exp table-set load during the DMA preamble
        escr = singles.tile([P, 1], F32)
        nc.scalar.activation(
            escr[:], ebias[:], mybir.ActivationFunctionType.Exp
        )

        # V' storage: [key-partition, key-tile, pair, dims]
        # even heads: cols 0:64 = V dims, col 64 = ones (denominator row 64)
        # odd heads: cols 64:128 = V dims, col 32 = ones (denominator row 32)
        qkvT = singles.tile([P, NFT, T], F16)
        eT = singles.tile([P, HPC // 2, T], F16)
        vpe = singles.tile([P, NKT, 2, D + 1], F16)
        vpo = singles.tile([P, NKT, 2, P], F16)
        nc.vector.memset(vpo[:, :, :, 0:D], 0.0)
        nc.gpsimd.dma_start(
            vpe[:, :, :, D : D + 1],
            bass.AP(tensor=vones_d[:].tensor, offset=vones_d[:].offset,
                    ap=[[NKT, P], [1, NKT], [0, 2]]),
        )
        nc.gpsimd.dma_start(
            vpo[:, :, :, 32:33],
            bass.AP(tensor=vones_d[:].tensor, offset=vones_d[:].offset,
                    ap=[[NKT, P], [1, NKT], [0, 2]]),
        )

        # --- emission helpers ---
        def emit_qk_proj_ft(c, ft):
            """one feature tile of the q,k projection for chunk c."""
            xc = xcs[c]
            pq = ps_mm.tile([P, QCH], F32, tag="mm512", name=f"pq{c}_{ft}")
            for ko in range(KO):
                nc.tensor.matmul(
                    pq[:],
                    w_sb[:, ko, ft * P : (ft + 1) * P],
                    xc[:, ko, :],
                    start=(ko == 0),
                    stop=(ko == KO - 1),
                )
            nc.vector.tensor_copy(
                qkvT[:, ft, c * QCH : (c + 1) * QCH], pq[:]
            )

        def emit_qk_proj(c):
            """q,k projection for chunk c -> qkvT[:, :, c*QCH:]."""
            for ft in range(NFT):
                emit_qk_proj_ft(c, ft)

        def emit_v_proj_tt(tt):
            """token-major V projection for one 128-token tile."""
            c, tl = tt // 4, tt % 4
            xc = xcs[c]
            vps = ps_mm.tile([P, HPC * D], F32, tag="mm512", name=f"vps{tt}")
            for ko in range(KO):
                nc.tensor.matmul(
                    vps[:],
                    xc[:, ko, tl * P : (tl + 1) * P],
                    wv_sb[:, ko, :],
                    start=(ko == 0),
                    stop=(ko == KO - 1),
                )
            # heads 2g -> vpe[:, tt, g, 0:64]; heads 2g+1 -> vpo[:, tt, g, 64:]
            nc.vector.tensor_copy(
                vpe[:, tt, :, 0:D],
                bass.AP(tensor=vps[:].tensor, offset=vps[:].offset,
                        ap=[[vps[:].ap[0][0], P], [2 * D, 2], [1, D]]),
            )
            nc.vector.tensor_copy(
                vpo[:, tt, :, D:P],
                bass.AP(tensor=vps[:].tensor,
                        offset=vps[:].offset + D,
                        ap=[[vps[:].ap[0][0], P], [2 * D, 2], [1, D]]),
            )

        def emit_outproj_tt(tt, eng=None):
            """partial output projection for one 128-token tile (all 1024
            output columns, one DMA)."""
            ot = otiles.tile([P, E], F16, tag="o")
            for oc in range(E // QCH):
                op_ps = ps_mm.tile(
                    [P, QCH], F32, tag="mm512", name=f"op{tt}_{oc}"
                )
                for g2 in range(HPC // 2):
                    nc.tensor.matmul(
                        op_ps[:],
                        eT[:, g2, tt * P : (tt + 1) * P],
                        w0_sb[:, g2, oc * QCH : (oc + 1) * QCH],
                        start=(g2 == 0),
                        stop=(g2 == HPC // 2 - 1),
                    )
                nc.vector.tensor_copy(
                    ot[:, oc * QCH : (oc + 1) * QCH], op_ps[:]
                )
            (eng or nc.sync).dma_start(out_d[tt * P : (tt + 1) * P, :], ot[:])

        norm_b_q = []  # deferred DVE halves of norms: [countdown, fn]

        def emit_norm_pair(g, c, pvs):
            """Evacuate both heads' PV psums and launch the denominators'
            DMA-bounce partition broadcasts (GPSIMD DMA queue — the DMA AXI
            port doesn't contend with PE's SBUF streaming, unlike a GPSIMD
            partition_broadcast). The DVE halves (reciprocal + multiply) are
            deferred so they never wait on the round trip at the head of the
            strict-FIFO vector queue."""
            sts, rbcs = [], []
            for r in range(2):
                rows = (D + 1) if r == 0 else P
                st = stage.tile([P, QCH], F32, tag="st", name=f"st{g}_{r}_{c}")
                nc.vector.tensor_copy(st[:rows, :], pvs[r][:rows, :])
                sts.append(st)
            for r in range(2):
                dp = D if r == 0 else 32
                db = dpool.tile([1, QCH], F32, name=f"db{g}_{r}_{c}")
                nc.gpsimd.dma_start(db[:], sts[r][dp : dp + 1, :])
                rbc = rtiles.tile([P, QCH], F32, tag="rbc")
                nc.gpsimd.dma_start(
                    rbc[:, :],
                    bass.AP(tensor=db[:].tensor, offset=db[:].offset,
                            ap=[[0, P], [1, QCH]]),
                )
                rbcs.append(rbc)

            def mk_norm_b(r):
                st, rbc = sts[r], rbcs[r]
                po = 64 * r

                def norm_b():
                    # custom-DVE reciprocal requires base_partition 0; run
                    # it on the full broadcast tile (cost is free-dim-bound)
                    nc.vector.reciprocal_approx_fast(rbc[:, :], rbc[:, :])
                    nc.vector.tensor_mul(
                        eT[po : po + 64, g, c * QCH : (c + 1) * QCH],
                        st[po : po + 64, :],
                        rbc[po : po + 64, :],
                    )

                return norm_b

            for r in range(2):
                norm_b_q.append([6, mk_norm_b(r)])

        def tick_norms():
            for e in norm_b_q:
                e[0] -= 1
            while norm_b_q and norm_b_q[0][0] <= 0:
                norm_b_q.pop(0)[1]()

        def drain_norms():
            while norm_b_q:
                norm_b_q.pop(0)[1]()

        # --- startup: chunk 0 q,k projection + V tiles run directly ---
        emit_qk_proj(0)
        for tt in range(4):
            emit_v_proj_tt(tt)

        # --- chunk-major attention with filler interleave ---
        fill_qkv = []   # must drain before the NEXT chunk's attention
        fill_any = []   # outproj fillers, emit whenever

        def pop_filler(reserve=0):
            if fill_qkv:
                fill_qkv.pop(0)()
            elif len(fill_any) > reserve:
                fill_any.pop(0)()

        for c in range(NCH):
            if c + 1 < NCH:
                dma_x_chunk(c + 1)
                # V tiles for chunk c+1 are produced early in attention(c+1)
                # itself (their first PV use is late in each pair's stream),
                # keeping PE filler available in the ACT-bound late chunks.

            for g in range(2):
                if c + 1 < NCH and g == 1:
                    # queue the next chunk's q,k projection only once its x
                    # prefetch has had half a chunk of attention to land
                    for ft in range(NFT):
                        fill_qkv.append(
                            lambda c=c, ft=ft: emit_qk_proj_ft(c + 1, ft)
                        )
                if c > 0 and g == 0:
                    for tt in range(4 * c, 4 * c + 4):
                        emit_v_proj_tt(tt)
                qt = [qkvT[64 * r : 64 * (r + 1), g, :] for r in range(2)]
                kt = [qkvT[64 * r : 64 * (r + 1), 2 + g, :] for r in range(2)]
                n_it = 4 * (c + 1)
                sps_buf = {}
                p_buf = {}
                pvs = [None, None]

                def emit_S(j):
                    off = 128 * (j - 4 * c) if j >= 4 * c else 0
                    sp = ps_s.tile(
                        [P, 2, QCH], F32, tag="s", name=f"s{g}_{j}"
                    )
                    for r in range(2):
                        nc.tensor.matmul(
                            sp[:, r, off:],
                            kt[r][:, j * P : (j + 1) * P],
                            qt[r][:, c * QCH + off : (c + 1) * QCH],
                            start=True,
                            stop=True,
                        )
                    sps_buf[j] = sp

                def emit_exp(j):
                    off = 128 * (j - 4 * c) if j >= 4 * c else 0
                    sp = sps_buf.pop(j)
                    p_t = ptiles.tile(
                        [P, 2, QCH], F16, tag="p", name=f"p{g}_{j}"
                    )
                    nc.scalar.activation(
                        p_t[:, :, off:], sp[:, :, off:],
                        mybir.ActivationFunctionType.Exp,
                        bias=ebias[:], scale=float(SCALE),
                    )
                    if j >= 4 * c:
                        for r in range(2):
                            nc.vector.tensor_mul(
                                p_t[:, r, off:], p_t[:, r, off:],
                                mask_sb[:, j - 4 * c, off:],
                            )
                    p_buf[j] = p_t

                def emit_PV(j):
                    off = 128 * (j - 4 * c) if j >= 4 * c else 0
                    p_t = p_buf.pop(j)
                    for r in range(2):
                        if j == 0:
                            pvs[r] = ps_pv.tile(
                                [P, QCH], F32, tag="pv", name=f"pv{g}_{r}"
                            )
                        rows = (D + 1) if r == 0 else P
                        vst = vpe[:, j, g, :] if r == 0 else vpo[:, j, g, :]
                        nc.tensor.matmul(
                            pvs[r][:rows, off:],
                            vst,
                            p_t[:, r, off:],
                            start=(j == 0),
                            stop=(j == n_it - 1),
                        )

                emit_S(0)
                emit_exp(0)
                if n_it > 1:
                    emit_S(1)
                    emit_exp(1)
                for i in range(n_it):
                    if i + 2 < n_it:
                        emit_S(i + 2)
                    emit_PV(i)
                    if i + 2 < n_it:
                        emit_exp(i + 2)
                    if i % 2 == 1:
                        pop_filler(
                            reserve={NCH - 2: 4, NCH - 1: 4}.get(c, 0)
                        )
                    tick_norms()
                emit_norm_pair(g, c, pvs)

            # next chunk's attention needs its q,k projection done; cover the
            # last norms' DMA round trip with fillers before their DVE half
            while fill_qkv:
                fill_qkv.pop(0)()
            npop = 0
            keep = 4 if c == NCH - 2 else 0
            nmax = 4 if c == NCH - 1 else 3
            while len(fill_any) > keep and norm_b_q and npop < nmax:
                fill_any.pop(0)()
                npop += 1
            if c == NCH - 1:
                # keep the HAM clock gate open while the last norms settle
                for w in range(12):
                    wps = ps_mm.tile(
                        [P, P], F32, tag="mm512", name=f"tailw{w}"
                    )
                    nc.tensor.matmul(
                        wps[:], wz[:], wz[:], start=True, stop=True
                    )
            drain_norms()
            # the last chunk's output DMAs drain at the kernel tail:
            # spread them over the idle engine queues
            tail_engs = [nc.sync, nc.gpsimd, nc.scalar, nc.sync]
            for k, tt in enumerate(range(4 * c, 4 * (c + 1))):
                eng = tail_engs[k] if c == NCH - 1 else nc.sync
                fill_any.append(
                    lambda tt=tt, eng=eng: emit_outproj_tt(tt, eng)
                )

        while fill_any:
            fill_any.pop(0)()


def make_inputs(x: np.ndarray, w_qkv: np.ndarray, w0: np.ndarray):
    """Build the 8 per-core input dicts (all host-packed for 2D DMAs)."""
    x = np.ascontiguousarray(np.asarray(x, dtype=np.float32)).reshape(B, S, E)
    w_qkv = np.ascontiguousarray(np.asarray(w_qkv, dtype=np.float32))
    w0 = np.ascontiguousarray(np.asarray(w0, dtype=np.float32))

    mask = np.zeros((P, 4, QCH), dtype=np.float16)
    f = np.arange(QCH)[None, :]
    p = np.arange(P)[:, None]
    for m in range(4):
        mask[:, m, :] = (f >= 128 * m + p).astype(np.float16)

    vones = np.ones((P, NKT), dtype=np.float16)

    # xR[p, c, ko, t] = x[b][c*512+t, ko*128+p]
    xR_b = []
    for b in range(B):
        xT = x[b].T.astype(np.float16)              # [E, T]
        xR = np.ascontiguousarray(
            xT.reshape(KO, P, NCH, QCH).transpose(1, 2, 0, 3)
        )
        xR_b.append(xR)

    in_maps = []
    for core in range(NCORES):
        b = core // CPB
        hb = (core % CPB) * HPC  # first head of this core
        rows = []
        for sec in range(2):  # q, k
            for g_ in range(HPC // 2):
                r0 = sec * E + (hb + 2 * g_) * D
                rows.append(w_qkv[r0 : r0 + 2 * D])
        w_slice = np.concatenate(rows, axis=0)  # [512, 1024]
        wT = w_slice.T.astype(np.float16)       # [1024, 512]
        # wT_packed[p, ko, f] = wT[ko*128+p, f]
        wTp = np.ascontiguousarray(wT.reshape(KO, P, NFT * P).transpose(1, 0, 2))

        v0 = 2 * E + hb * D
        wvT = w_qkv[v0 : v0 + HPC * D].T.astype(np.float16)  # [1024, 256]
        wvTp = np.ascontiguousarray(
            wvT.reshape(KO, P, HPC * D).transpose(1, 0, 2)
        )

        w0T = np.empty((P, HPC // 2, E), dtype=np.float16)
        for g_ in range(HPC // 2):
            cols = slice((hb + 2 * g_) * D, (hb + 2 * g_ + 2) * D)
            w0T[:, g_, :] = w0[:, cols].T.astype(np.float16)
        in_maps.append(
            {
                "xR": xR_b[b],
                "wT": wTp,
                "wvT": wvTp,
                "w0T": w0T,
                "mask": mask,
                "vones": vones,
            }
        )
    return in_maps


_NC_CACHE = None


def kernel(x, w_qkv, w0, trace=False, trace_cores=None):
    global _NC_CACHE
    if _NC_CACHE is None:
        _NC_CACHE = build_nc()
    nc = _NC_CACHE
    in_maps = make_inputs(x, w_qkv, w0)
    res = run_bass_kernel_spmd(
        nc, in_maps, list(range(NCORES)), trace=trace, trace_cores=trace_cores
    )
    kernel.last_results = res
    outs = [res.results[c]["out"] for c in range(NCORES)]
    full = np.empty((B, S, E), dtype=np.float32)
    for b in range(B):
        full[b] = np.sum(
            [outs[i].astype(np.float32) for i in range(b * CPB, (b + 1) * CPB)],
            axis=0,
        )
    return full



# revision 53
# speedup vs baseline: 1.0374x; 1.0374x over previous
"""Multi-head self-attention (B=2, S=2048, E=1024, H=16, D=64) on 8 TRN2 cores.

Sharding: core c handles batch b = c // 4 and heads (c % 4)*4 .. +4 (data
parallel over batch x tensor parallel over heads). Each core computes a
partial output projection over its 4 heads' slice of the residual; the host
sums the 4 fp16 partials per batch in fp32.

v4 changes over v3:
  - Causal mask is applied additively (-6e4) on the scores PSUM before the
    exp, restricted to the 128-column diagonal triangle window. p_t then has
    a single producer (ACT), so PV matmuls carry one wait and their
    LDWEIGHTS prefetch under the previous matmul instead of serializing.
  - vpe padded to 128 stationary columns (zeros beyond the ones column) so
    the even-head PV LDWEIGHTS takes the 4-way fast-weight-load path.
  - Startup: per-ko DMA pieces in consumption order over 4 queues; chunk-0
    q01/k01 projections run ko-major with zero-matmul filler so the PE HAM
    activity window stays busy and the clock un-throttles ~5us in.
  - Last-chunk norms broadcast the denominator row via a one-hot stationary
    matmul instead of the DRAM-bounce DMA, collapsing the kernel tail.

v3 kernel structure (chunk-major flash pipeline):
  - fp16 matmul datapath, fp32 PSUM accumulation, exp with -4 bias.
  - QKV projection is chunk-major: attention for query-chunk c starts as soon
    as projections for chunks <= c are done; projections for chunk c+1 and
    output-projection tiles are interleaved into the attention stream as PE
    filler, which also keeps the HAM clock gate open through the ACT-bound
    stretches.
  - V is projected token-major directly (x tile stationary, w_v moving): no
    PE transposes. V tiles for chunk c are produced early in attention(c).
  - Score matmuls have K=64 (head dim): the two heads of a pair live at
    partitions 0-63 / 64-127, so their score matmuls auto-derive PE row-tile
    positions (0,0)/(64,0) and execute concurrently on the array. Both heads
    score into ONE two-bank psum tile so a single ACTIVATE does both exps
    (the ~352-cycle ACTIVATE overhead is the scalar-engine tax).
  - Causality at 128-column granularity everywhere: scores, exp, and the PV
    accumulation are restricted to [off:] on diagonal key tiles.
  - Softmax denominator comes out of the PV matmul for free (ones column in
    the stationary). reciprocal_approx_fast + DRAM-bounce partition
    broadcast, deferred so the vector queue never waits on the round trip.
    Broadcast/output DMAs are triggered from the idle GPSIMD queue.
  - All weight/activation DMAs are host-packed so each is one plain 2D
    descriptor (descriptor generation on the Sync queue was a startup
    bottleneck).
"""

import sys
import types

import numpy as np


def _ensure_ntff_hook():
    """Install the axon NTFF profiling hook if the image's antenv lacks it."""
    try:
        import antenv.axon_hooks  # noqa: F401

        return
    except ImportError:
        pass
    try:
        import antenv
    except ImportError:
        return
    mod = types.ModuleType("antenv.axon_hooks")
    state = {"hook": None}
    mod.set_axon_ntff_profile_hook = lambda h: state.__setitem__("hook", h)
    mod.get_axon_ntff_profile_hook = lambda: state["hook"]
    sys.modules["antenv.axon_hooks"] = mod
    antenv.axon_hooks = mod
    try:
        from trn_agent_boot.trn_boot import _ntff_profile_via_ctypes

        mod.set_axon_ntff_profile_hook(
            _ntff_profile_via_ctypes("/opt/axon/libaxon_pjrt.so")
        )
    except Exception:
        pass


_ensure_ntff_hook()

import concourse.bass as bass
import concourse.mybir as mybir
import concourse.tile as tile
from concourse import bacc
from concourse import bass_utils as _bass_utils
from concourse.bass_utils import run_bass_kernel_spmd

_orig_upload = _bass_utils.upload_artifacts


def _safe_upload(tmpdir):
    try:
        return _orig_upload(tmpdir)
    except Exception:
        return ""


_bass_utils.upload_artifacts = _safe_upload

F32 = mybir.dt.float32
F16 = mybir.dt.float16

B, S, E, H, D = 2, 2048, 1024, 16, 64
NCORES = 8
CPB = 4            # cores per batch
HPC = H // CPB     # heads per core = 4
T = S              # tokens per core (one batch)
P = 128
QCH = 512          # query-chunk (psum free dim)
NCH = T // QCH     # 4 query chunks
NKT = T // P       # 16 key tiles
KO = E // P        # 8 contraction subtiles for the projections
NFT = 4            # q01, q23, k01, k23 feature tiles (v is token-major)
SCALE = 1.0 / np.sqrt(D)
EXP_BIAS = -4.0    # keeps exp() in fp16 range; cancels in softmax


def build_nc() -> bass.Bass:
    nc = bacc.Bacc(None, target_bir_lowering=False)
    xR_d = nc.dram_tensor("xR", [P, NCH, KO, QCH], F16, kind="ExternalInput")
    wT_d = nc.dram_tensor("wT", [P, KO, NFT * P], F16, kind="ExternalInput")
    wvT_d = nc.dram_tensor("wvT", [P, KO, HPC * D], F16, kind="ExternalInput")
    w0T_d = nc.dram_tensor("w0T", [P, HPC // 2, E], F16, kind="ExternalInput")
    mask_d = nc.dram_tensor("mask", [P, P], F16, kind="ExternalInput")
    vinit_d = nc.dram_tensor("vinit", [P, NKT], F16, kind="ExternalInput")
    out_d = nc.dram_tensor("out", [T, E], F16, kind="ExternalOutput")

    with tile.TileContext(nc) as tc, nc.allow_low_precision(
        reason="fp16 matmul datapath by design; fp32 PSUM accumulation"
    ):
        _body(tc, xR_d, wT_d, wvT_d, w0T_d, mask_d, vinit_d, out_d)
    nc.finalize()
    return nc


def _body(tc, xR_d, wT_d, wvT_d, w0T_d, mask_d, vinit_d, out_d):
    nc = tc.nc
    with (
        tc.tile_pool(name="singles", bufs=1) as singles,
        tc.tile_pool(name="xchunks", bufs=2) as xchunks,
        tc.tile_pool(name="ptiles", bufs=4) as ptiles,
        tc.tile_pool(name="rtiles", bufs=4) as rtiles,
        tc.tile_pool(name="otiles", bufs=4) as otiles,
        tc.tile_pool(name="dpool", bufs=8, space="DRAM") as dpool,
        tc.tile_pool(name="stage", bufs=4) as stage,
        tc.tile_pool(name="ps_mm", bufs=2, space="PSUM") as ps_mm,
        tc.tile_pool(name="ps_s", bufs=2, space="PSUM") as ps_s,
        tc.tile_pool(name="ps_pv", bufs=2, space="PSUM") as ps_pv,
    ):
        # zero tile for HAM-warming filler matmuls (DVE memset is the only
        # dependency); warmup psums go to the ps_pv pool, which has no real
        # user until the first attention PV, so fillers never collide with
        # the open projection accumulation chains in ps_mm.
        wz = singles.tile([P, P], F16)
        nc.vector.memset(wz[:], 0.0)
        nwarm = [0]

        def emit_warm(pool=None):
            # startup fillers use ps_pv (no PV user yet); attention-time
            # fillers must use ps_mm instead, since ps_pv's banks hold the
            # open PV accumulation pair
            wps = (pool or ps_pv).tile(
                [P, QCH], F32, tag="pv" if pool is None else "mm512",
                name=f"warm{nwarm[0]}",
            )
            nwarm[0] += 1
            nc.tensor.matmul(wps[:, :P], wz[:], wz[:], start=True, stop=True)

        for _ in range(8):
            emit_warm()

        # preamble DMAs are split into per-ko pieces, interleaved in the
        # order the first projection chains consume them, over four
        # DMA-capable engine queues.
        w_sb = singles.tile([P, KO, NFT * P], F16)
        wv_sb = singles.tile([P, KO, HPC * D], F16)
        xcs = [None] * NCH
        xc0 = xchunks.tile([P, KO, QCH], F16, tag="xc", name="xc0")
        xcs[0] = xc0
        engs = [nc.sync, nc.scalar, nc.gpsimd]
        for i in range(KO):
            engs[(2 * i) % 3].dma_start(w_sb[:, i : i + 1, :], wT_d[:, i : i + 1, :])
            engs[(2 * i + 1) % 3].dma_start(
                xc0[:, i : i + 1, :], xR_d[:, 0, i : i + 1, :]
            )
        nc.sync.dma_start(wv_sb[:, : KO // 2, :], wvT_d[:, : KO // 2, :])
        nc.scalar.dma_start(wv_sb[:, KO // 2 :, :], wvT_d[:, KO // 2 :, :])
        mask_sb = singles.tile([P, P], F16)
        nc.gpsimd.dma_start(mask_sb[:], mask_d[:])
        w0_sb = singles.tile([P, HPC // 2, E], F16)

        def dma_x_chunk(c):
            xc = xchunks.tile([P, KO, QCH], F16, tag="xc", name=f"xc{c}")
            nc.sync.dma_start(xc[:, : KO // 2, :], xR_d[:, c, : KO // 2, :])
            nc.scalar.dma_start(xc[:, KO // 2 :, :], xR_d[:, c, KO // 2 :, :])
            xcs[c] = xc

        ebias = singles.tile([P, 1], F32)
        nc.vector.memset(ebias[:], EXP_BIAS)
        # trigger the exp table-set load during the DMA preamble
        escr = singles.tile([P, 1], F32)
        nc.scalar.activation(
            escr[:], ebias[:], mybir.ActivationFunctionType.Exp
        )

        # V' storage: [key-partition, key-tile, pair, dims]
        # even heads: cols 0:64 = V dims, col 64 = ones (denominator row 64),
        #   cols 65:128 zero-padded so the stationary is a full 128 columns
        #   (enables the 4-way fast weight load path on its LDWEIGHTS)
        # odd heads: cols 64:128 = V dims, col 32 = ones (denominator row 32)
        qkvT = singles.tile([P, NFT, T], F16)
        eT = singles.tile([P, HPC // 2, T], F16)
        vpe = singles.tile([P, NKT, 2, P], F16)
        vpo = singles.tile([P, NKT, 2, P], F16)
        # the zero padding is DVE-memset during the otherwise idle preamble
        # window; the ones columns come from a tiny stride-replicated DMA.
        # (A host-packed init tensor would cost 1MB of the DMA-saturated
        # startup window and delay the chunk-1 x prefetch.)
        nc.vector.memset(vpe[:, :, :, D + 1 :], 0.0)
        nc.vector.memset(vpo[:, :, :, 0:D], 0.0)
        nc.gpsimd.dma_start(
            vpe[:, :, :, D : D + 1],
            bass.AP(tensor=vinit_d[:].tensor, offset=vinit_d[:].offset,
                    ap=[[NKT, P], [1, NKT], [0, 2]]),
        )
        nc.gpsimd.dma_start(
            vpo[:, :, :, 32:33],
            bass.AP(tensor=vinit_d[:].tensor, offset=vinit_d[:].offset,
                    ap=[[NKT, P], [1, NKT], [0, 2]]),
        )

        # one-hot broadcast stationaries for the last-chunk norm path:
        # matmul(out, bce, st) replicates st's denominator row (64 / 32)
        # across all 128 output partitions without a DMA round trip.
        bce = singles.tile([P, P], F16)
        bco = singles.tile([P, P], F16)
        nc.vector.memset(bce[:], 0.0)
        nc.vector.memset(bco[:], 0.0)
        nc.vector.memset(bce[D : D + 1, :], 1.0)
        nc.vector.memset(bco[32:33, :], 1.0)

        # --- emission helpers ---
        def emit_qk_proj_ft(c, ft):
            """one feature tile of the q,k projection for chunk c."""
            xc = xcs[c]
            pq = ps_mm.tile([P, QCH], F32, tag="mm512", name=f"pq{c}_{ft}")
            for ko in range(KO):
                nc.tensor.matmul(
                    pq[:],
                    w_sb[:, ko, ft * P : (ft + 1) * P],
                    xc[:, ko, :],
                    start=(ko == 0),
                    stop=(ko == KO - 1),
                )
            nc.vector.tensor_copy(
                qkvT[:, ft, c * QCH : (c + 1) * QCH], pq[:]
            )

        def emit_qk_proj(c):
            """q,k projection for chunk c -> qkvT[:, :, c*QCH:]."""
            for ft in range(NFT):
                emit_qk_proj_ft(c, ft)

        def emit_qk_proj_startup():
            """chunk-0 q,k projection: the two feature tiles attention pair
            g=0 needs (q01, k01) run first, ko-major so each per-ko DMA
            piece is consumed as it lands, with zero-matmul filler keeping
            the PE (and its HAM activity window) busy during the DMA gaps."""
            fts = (0, 2)
            pqs = {
                ft: ps_mm.tile([P, QCH], F32, tag="mm512", name=f"pq0_{ft}")
                for ft in fts
            }
            for ko in range(KO):
                for ft in fts:
                    nc.tensor.matmul(
                        pqs[ft][:],
                        w_sb[:, ko, ft * P : (ft + 1) * P],
                        xc0[:, ko, :],
                        start=(ko == 0),
                        stop=(ko == KO - 1),
                    )
                if ko < KO - 1:
                    emit_warm()
                    emit_warm()
            for ft in fts:
                nc.vector.tensor_copy(qkvT[:, ft, 0:QCH], pqs[ft][:])
            emit_qk_proj_ft(0, 1)
            emit_qk_proj_ft(0, 3)

        def emit_v_proj_tt(tt):
            """token-major V projection for one 128-token tile."""
            c, tl = tt // 4, tt % 4
            xc = xcs[c]
            vps = ps_mm.tile([P, HPC * D], F32, tag="mm512", name=f"vps{tt}")
            for ko in range(KO):
                nc.tensor.matmul(
                    vps[:],
                    xc[:, ko, tl * P : (tl + 1) * P],
                    wv_sb[:, ko, :],
                    start=(ko == 0),
                    stop=(ko == KO - 1),
                )
            # heads 2g -> vpe[:, tt, g, 0:64]; heads 2g+1 -> vpo[:, tt, g, 64:]
            nc.vector.tensor_copy(
                vpe[:, tt, :, 0:D],
                bass.AP(tensor=vps[:].tensor, offset=vps[:].offset,
                        ap=[[vps[:].ap[0][0], P], [2 * D, 2], [1, D]]),
            )
            nc.vector.tensor_copy(
                vpo[:, tt, :, D:P],
                bass.AP(tensor=vps[:].tensor,
                        offset=vps[:].offset + D,
                        ap=[[vps[:].ap[0][0], P], [2 * D, 2], [1, D]]),
            )

        def emit_outproj_tt(tt, eng=None):
            """partial output projection for one 128-token tile (all 1024
            output columns, one DMA)."""
            ot = otiles.tile([P, E], F16, tag="o")
            for oc in range(E // QCH):
                op_ps = ps_mm.tile(
                    [P, QCH], F32, tag="mm512", name=f"op{tt}_{oc}"
                )
                for g2 in range(HPC // 2):
                    nc.tensor.matmul(
                        op_ps[:],
                        eT[:, g2, tt * P : (tt + 1) * P],
                        w0_sb[:, g2, oc * QCH : (oc + 1) * QCH],
                        start=(g2 == 0),
                        stop=(g2 == HPC // 2 - 1),
                    )
                nc.vector.tensor_copy(
                    ot[:, oc * QCH : (oc + 1) * QCH], op_ps[:]
                )
            (eng or nc.sync).dma_start(out_d[tt * P : (tt + 1) * P, :], ot[:])

        def emit_norm_pair_pe(g, c, pvs):
            """Final-pair norm path: evacuate PV psums to fp16 stage tiles,
            broadcast each denominator row across partitions with a one-hot
            stationary matmul (no DMA round trip), reciprocal on DVE, scale
            into eT. The broadcast psums reuse the ps_pv banks the PV pair
            vacated — there is no later PV pair to gate."""
            sts = []
            for r in range(2):
                rows = (D + 1) if r == 0 else P
                st = stage.tile([P, QCH], F16, tag="st16", name=f"stf{g}_{r}_{c}")
                nc.vector.tensor_copy(st[:rows, :], pvs[r][:rows, :])
                sts.append(st)
            for r in range(2):
                # g=0's broadcasts use ps_mm so they don't gate g=1's PV
                # allocation; g=1 (the very last pair) reuses ps_pv, which
                # has no successor
                pool, tg = (ps_mm, "mm512") if g == 0 else (ps_pv, "pv")
                rbc = pool.tile([P, QCH], F32, tag=tg, name=f"rbc{g}_{r}_{c}")
                if r == 0:
                    # garbage PV rows 65:128 are excluded from the moving
                    # operand so NaNs can't leak through 0-weight products
                    nc.tensor.matmul(
                        rbc[:], bce[: D + 1, :], sts[0][: D + 1, :],
                        start=True, stop=True,
                    )
                else:
                    nc.tensor.matmul(
                        rbc[:], bco[:], sts[1][:], start=True, stop=True
                    )
                rr = rtiles.tile([P, QCH], F32, tag="rbc", name=f"rr{g}_{r}_{c}")
                nc.vector.reciprocal_approx_fast(rr[:], rbc[:])
                po = 64 * r
                nc.vector.tensor_mul(
                    eT[po : po + 64, g, c * QCH : (c + 1) * QCH],
                    sts[r][po : po + 64, :],
                    rr[po : po + 64, :],
                )

        norm_b_q = []  # deferred DVE halves of norms: [countdown, fn]

        def emit_norm_pair(g, c, pvs):
            """Evacuate both heads' PV psums and launch the denominators'
            DMA-bounce partition broadcasts (GPSIMD DMA queue). The DVE
            halves (reciprocal + multiply) are deferred so they never wait
            on the round trip at the head of the strict-FIFO vector queue,
            and no psum bank is held across the deferral."""
            sts, rbcs = [], []
            for r in range(2):
                rows = (D + 1) if r == 0 else P
                st = stage.tile([P, QCH], F32, tag="st", name=f"st{g}_{r}_{c}")
                nc.vector.tensor_copy(st[:rows, :], pvs[r][:rows, :])
                sts.append(st)
            for r in range(2):
                dp = D if r == 0 else 32
                db = dpool.tile([1, QCH], F32, name=f"db{g}_{r}_{c}")
                nc.gpsimd.dma_start(db[:], sts[r][dp : dp + 1, :])
                rbc = rtiles.tile([P, QCH], F32, tag="rbc")
                nc.gpsimd.dma_start(
                    rbc[:, :],
                    bass.AP(tensor=db[:].tensor, offset=db[:].offset,
                            ap=[[0, P], [1, QCH]]),
                )
                rbcs.append(rbc)

            def mk_norm_b(r):
                st, rbc = sts[r], rbcs[r]
                po = 64 * r

                def norm_b():
                    # custom-DVE reciprocal requires base_partition 0; run
                    # it on the full broadcast tile (cost is free-dim-bound)
                    nc.vector.reciprocal_approx_fast(rbc[:, :], rbc[:, :])
                    nc.vector.tensor_mul(
                        eT[po : po + 64, g, c * QCH : (c + 1) * QCH],
                        st[po : po + 64, :],
                        rbc[po : po + 64, :],
                    )

                return norm_b

            for r in range(2):
                norm_b_q.append([6, mk_norm_b(r)])

        def tick_norms():
            for e in norm_b_q:
                e[0] -= 1
            while norm_b_q and norm_b_q[0][0] <= 0:
                norm_b_q.pop(0)[1]()

        def drain_norms():
            while norm_b_q:
                norm_b_q.pop(0)[1]()

        # --- startup: chunk 0 q,k projection + V tiles run directly ---
        emit_qk_proj_startup()
        for tt in range(4):
            emit_v_proj_tt(tt)

        # --- chunk-major attention with filler interleave ---
        fill_qkv = []   # must drain before the NEXT chunk's attention
        fill_any = []   # outproj fillers, emit whenever

        def pop_filler(reserve=0):
            if fill_qkv:
                fill_qkv.pop(0)()
            elif len(fill_any) > reserve:
                fill_any.pop(0)()

        for c in range(NCH):
            if c + 1 < NCH:
                dma_x_chunk(c + 1)
                # V tiles for chunk c+1 are produced early in attention(c+1)
                # itself (their first PV use is late in each pair's stream),
                # keeping PE filler available in the ACT-bound late chunks.

            for g in range(2):
                if c + 1 < NCH and g == 1:
                    # queue the next chunk's q,k projection only once its x
                    # prefetch has had half a chunk of attention to land
                    for ft in range(NFT):
                        fill_qkv.append(
                            lambda c=c, ft=ft: emit_qk_proj_ft(c + 1, ft)
                        )
                if c > 0 and g == 0:
                    for tt in range(4 * c, 4 * c + 4):
                        emit_v_proj_tt(tt)
                qt = [qkvT[64 * r : 64 * (r + 1), g, :] for r in range(2)]
                kt = [qkvT[64 * r : 64 * (r + 1), 2 + g, :] for r in range(2)]
                n_it = 4 * (c + 1)
                sps_buf = {}
                p_buf = {}
                pvs = [None, None]

                def emit_S(j):
                    off = 128 * (j - 4 * c) if j >= 4 * c else 0
                    sp = ps_s.tile(
                        [P, 2, QCH], F32, tag="s", name=f"s{g}_{j}"
                    )
                    for r in range(2):
                        nc.tensor.matmul(
                            sp[:, r, off:],
                            kt[r][:, j * P : (j + 1) * P],
                            qt[r][:, c * QCH + off : (c + 1) * QCH],
                            start=True,
                            stop=True,
                        )
                    sps_buf[j] = sp

                def emit_mask(j):
                    """additive causal mask (-6e4 above the diagonal) on the
                    scores psum BEFORE the exp, restricted to the 128-column
                    triangle window. Applying it pre-exp keeps p_t a single-
                    producer tile (ACT only), so the PV matmuls carry one
                    wait and their LDWEIGHTS can prefetch during the
                    previous matmul instead of serializing behind the mask
                    semaphore."""
                    if j < 4 * c:
                        return
                    off = 128 * (j - 4 * c)
                    sp = sps_buf[j]
                    for r in range(2):
                        nc.vector.tensor_add(
                            sp[:, r, off : off + P],
                            sp[:, r, off : off + P],
                            mask_sb[:, :],
                        )

                def emit_exp(j):
                    off = 128 * (j - 4 * c) if j >= 4 * c else 0
                    sp = sps_buf.pop(j)
                    p_t = ptiles.tile(
                        [P, 2, QCH], F16, tag="p", name=f"p{g}_{j}"
                    )
                    nc.scalar.activation(
                        p_t[:, :, off:], sp[:, :, off:],
                        mybir.ActivationFunctionType.Exp,
                        bias=ebias[:], scale=float(SCALE),
                    )
                    p_buf[j] = p_t

                def emit_PV(j):
                    off = 128 * (j - 4 * c) if j >= 4 * c else 0
                    p_t = p_buf.pop(j)
                    for r in range(2):
                        if j == 0:
                            pvs[r] = ps_pv.tile(
                                [P, QCH], F32, tag="pv", name=f"pv{g}_{r}"
                            )
                        vst = vpe[:, j, g, :] if r == 0 else vpo[:, j, g, :]
                        nc.tensor.matmul(
                            pvs[r][:, off:],
                            vst,
                            p_t[:, r, off:],
                            start=(j == 0),
                            stop=(j == n_it - 1),
                        )

                emit_S(0)
                emit_mask(0)
                emit_exp(0)
                if n_it > 1:
                    emit_S(1)
                    emit_mask(1)
                    emit_exp(1)
                for i in range(n_it):
                    if i + 2 < n_it:
                        emit_S(i + 2)
                        emit_mask(i + 2)
                    emit_PV(i)
                    if i + 2 < n_it:
                        emit_exp(i + 2)
                    if i % 2 == 1:
                        pop_filler(
                            reserve={NCH - 2: 4}.get(c, 0)
                        )
                    tick_norms()
                if c == NCH - 1:
                    emit_norm_pair_pe(g, c, pvs)
                else:
                    emit_norm_pair(g, c, pvs)

            # next chunk's attention needs its q,k projection done; cover
            # the last norms' DMA round trip with fillers before their DVE
            # half
            while fill_qkv:
                fill_qkv.pop(0)()
            if c < NCH - 1:
                npop = 0
                keep = 4 if c == NCH - 2 else 0
                while len(fill_any) > keep and norm_b_q and npop < 3:
                    fill_any.pop(0)()
                    npop += 1
                drain_norms()
            if c == 0:
                # w0 is first needed by chunk 0's outproj fillers, which
                # pop during chunk 1; loading it here keeps its 0.5MB out
                # of the DMA-saturated startup window
                nc.gpsimd.dma_start(w0_sb[:], w0T_d[:])
            # the last chunk's output DMAs drain at the kernel tail:
            # spread them over the idle engine queues
            tail_engs = [nc.sync, nc.gpsimd, nc.scalar, nc.sync]
            for k, tt in enumerate(range(4 * c, 4 * (c + 1))):
                eng = tail_engs[k] if c == NCH - 1 else nc.sync
                fill_any.append(
                    lambda tt=tt, eng=eng: emit_outproj_tt(tt, eng)
                )

        while fill_any:
            fill_any.pop(0)()


def make_inputs(x: np.ndarray, w_qkv: np.ndarray, w0: np.ndarray):
    """Build the 8 per-core input dicts (all host-packed for 2D DMAs)."""
    x = np.ascontiguousarray(np.asarray(x, dtype=np.float32)).reshape(B, S, E)
    w_qkv = np.ascontiguousarray(np.asarray(w_qkv, dtype=np.float32))
    w0 = np.ascontiguousarray(np.asarray(w0, dtype=np.float32))

    # additive causal mask for the 128x128 diagonal triangle window:
    # column w (query off+w) may attend key partition p iff w >= p
    w_ix = np.arange(P)[None, :]
    p_ix = np.arange(P)[:, None]
    mask = np.where(w_ix >= p_ix, np.float16(0.0), np.float16(-60000.0))
    mask = np.ascontiguousarray(mask.astype(np.float16))

    vinit = np.ones((P, NKT), dtype=np.float16)

    # xR[p, c, ko, t] = x[b][c*512+t, ko*128+p]
    xR_b = []
    for b in range(B):
        xT = x[b].T.astype(np.float16)              # [E, T]
        xR = np.ascontiguousarray(
            xT.reshape(KO, P, NCH, QCH).transpose(1, 2, 0, 3)
        )
        xR_b.append(xR)

    in_maps = []
    for core in range(NCORES):
        b = core // CPB
        hb = (core % CPB) * HPC  # first head of this core
        rows = []
        for sec in range(2):  # q, k
            for g_ in range(HPC // 2):
                r0 = sec * E + (hb + 2 * g_) * D
                rows.append(w_qkv[r0 : r0 + 2 * D])
        w_slice = np.concatenate(rows, axis=0)  # [512, 1024]
        wT = w_slice.T.astype(np.float16)       # [1024, 512]
        # wT_packed[p, ko, f] = wT[ko*128+p, f]
        wTp = np.ascontiguousarray(wT.reshape(KO, P, NFT * P).transpose(1, 0, 2))

        v0 = 2 * E + hb * D
        wvT = w_qkv[v0 : v0 + HPC * D].T.astype(np.float16)  # [1024, 256]
        wvTp = np.ascontiguousarray(
            wvT.reshape(KO, P, HPC * D).transpose(1, 0, 2)
        )

        w0T = np.empty((P, HPC // 2, E), dtype=np.float16)
        for g_ in range(HPC // 2):
            cols = slice((hb + 2 * g_) * D, (hb + 2 * g_ + 2) * D)
            w0T[:, g_, :] = w0[:, cols].T.astype(np.float16)
        in_maps.append(
            {
                "xR": xR_b[b],
                "wT": wTp,
                "wvT": wvTp,
                "w0T": w0T,
                "mask": mask,
                "vinit": vinit,
            }
        )
    return in_maps


_NC_CACHE = None


def kernel(x, w_qkv, w0, trace=False, trace_cores=None):
    global _NC_CACHE
    if _NC_CACHE is None:
        _NC_CACHE = build_nc()
    nc = _NC_CACHE
    in_maps = make_inputs(x, w_qkv, w0)
    res = run_bass_kernel_spmd(
        nc, in_maps, list(range(NCORES)), trace=trace, trace_cores=trace_cores
    )
    kernel.last_results = res
    outs = [res.results[c]["out"] for c in range(NCORES)]
    full = np.empty((B, S, E), dtype=np.float32)
    for b in range(B):
        full[b] = np.sum(
            [outs[i].astype(np.float32) for i in range(b * CPB, (b + 1) * CPB)],
            axis=0,
        )
    return full

